# revision 9
# baseline (speedup 1.0000x reference)
"""Trainium2 Bass kernel: multi-head attention (B=4, S=2048, E=1024, H=16, D=64).

Sharding: 8 cores = 4 batches x 2 head-groups (8 heads each). Each core
computes attention for its (batch, 8-head group) and a partial output
projection over its 512 channels; the host sums the two partials per batch
and adds the output bias.

Per-core dataflow (bf16 matmuls, fp32 PSUM):
  A[j] = wA[j].T @ XT_aug[j]    -> [Q'^T (rows 0:64) ; K^T (rows 64:128)]
         where Q' = (x Wq + bq) * log2(e)/32  (scale folded on host)
  B[j] = partition-swapped copy of A[j] (SBUF->SBUF DMA) = [K^T ; Q'^T]
  scores^T for two t-tiles at once via PE row tiling (K=64):
         T0 tile (0,0):  B[0:64,tcols].T @ A[0:64,s]   (t even slot)
         T8 tile (64,0): A[64:128,tcols].T @ B[64:128,s] (t odd slot)
  psum w = scores * log2(e)/32; exp(s/8) = 2^(4w):
         ScalarE groups: Exp activation, scale = 4*ln2
         DVE groups:     custom 8-stage op (cubic 2^w then 2 squarings);
                         global scale a0^4 cancels in softmax
  attnV: av[65,512] += V_aug[tt].T @ exp_tile  (ones col -> denominator)
  normalize: recip via DMA-bounce broadcast, multiply on GpSimd
  out partial = concatT.T @ WoT -> [s,1024] fp32 -> DRAM
"""

import sys

sys.path.insert(0, "/opt/trn_rl_repo")

import numpy as np
import ml_dtypes

BF16 = ml_dtypes.bfloat16

B, S, E, H = 4, 2048, 1024, 16
D = E // H          # 64
HL = 8              # heads per core
N_CORES = 8
NT = S // 128       # 16 t-tiles
NC_CHUNK = 4        # s-chunks of 512
SIG = float(np.log2(np.e) / 64.0)   # score scale folded into Wq

# custom DVE op: P(w)=((c3 w + c2) w + c1) w + 1 ; out = (P^2)^2 ~ 2^(4w)/a0^4
# kernel folds SIG=log2e/64 into Wq, so DVE out = exp(s/16)/a0^4; a final
# gpsimd square yields exp(s/8)/a0^8 (the a0^8 cancels in softmax).
EXP4_NAME = "ANT_EXP4_SOFTMAX"
EXP4_C3 = 0.0558784277
EXP4_C2 = 0.242307174
EXP4_C1 = 0.693163145
EXP4_A0 = 0.99994823
EXP4_SCALE = EXP4_A0 ** 4   # DVE output is 2^(4w)/EXP4_SCALE

# group structure per s-chunk: (ntiles, region, engine)
# regions: 'P' (banks 0-3, [128,2048]), 'Q' (banks 4-6, [128,1536]),
#          'P2' ([128,1024] from P pool)
GROUPS = [
    (4, "P", "act"),    # t0-3
    (3, "Q", "act"),    # t4-6
    (4, "P", "act"),    # t7-10
    (3, "Q", "dve"),    # t11-13
    (2, "P2", "dve"),   # t14,15
]

_CACHE = {}


def register_exp4():
    import concourse.dve_ops as dmod
    from concourse.dve_spec import Spec, Src0, C0, C1, C2, One, sq, lower
    from concourse.dve_uop import DveOpSpec

    for op in dmod.OPS:
        if op.name == EXP4_NAME:
            return op

    body = sq(sq(((C0 * Src0 + C1) * Src0 + C2) * Src0 + One))

    def _ref(in0, in1, s0, s1, imm2):
        x = in0.astype(np.float32)
        p = (((s0 * x + s1) * x + imm2) * x + np.float32(1.0)).astype(np.float32)
        p2 = (p * p).astype(np.float32)
        return (p2 * p2).astype(np.float32)

    spec = Spec(body=body, reference=_ref)
    row = dmod._CUSTOM_DVE_ROW_BASE + len(dmod.OPS)
    assert row < 0x20
    shas = {}
    for ver in ("v3", "v4"):
        try:
            uops = lower(spec, ver=ver)
            shas[ver] = DveOpSpec(
                name=EXP4_NAME, opcode=row, uops=uops, rd1_en=False
            ).sha(ver)
        except Exception:
            pass
    op = dmod.DveOp(EXP4_NAME, spec, subdim=False, uops_sha=shas)
    dmod.OPS.append(op)
    dmod.CUSTOM_DVE_SPECS[EXP4_NAME] = spec
    dmod._SUB_OPCODE_FOR_NAME[EXP4_NAME] = row
    return op


def build_nc():
    import concourse.mybir as mybir
    import concourse.tile as tile
    from concourse import bacc

    f32 = mybir.dt.float32
    bf16 = mybir.dt.bfloat16
    exp4 = register_exp4()
    ACT_SCALE = float(8.0 * np.log(2.0))

    nc = bacc.Bacc(None)

    xt_d = nc.dram_tensor("xt", [HL, 128, S], bf16, kind="ExternalInput")
    wa_d = nc.dram_tensor("wa", [HL, 128, 128], bf16, kind="ExternalInput")
    wv_d = nc.dram_tensor("wv", [HL, 128, D + 1], bf16, kind="ExternalInput")
    wot_d = nc.dram_tensor("wot", [HL * D, E], bf16, kind="ExternalInput")
    out_d = nc.dram_tensor("out", [S, E], f32, kind="ExternalOutput")
    recip_d = nc.dram_tensor("recip_dram", [HL, S], f32)

    with tile.TileContext(nc) as tc:
        with (
            tc.tile_pool(name="xt", bufs=3) as xt_pool,
            tc.tile_pool(name="w", bufs=2 * HL) as w_pool,
            tc.tile_pool(name="ab", bufs=2 * HL) as ab_pool,
            tc.tile_pool(name="v", bufs=HL) as v_pool,
            tc.tile_pool(name="wot", bufs=4) as wot_pool,
            tc.tile_pool(name="eg", bufs=4) as eg_pool,
            tc.tile_pool(name="eh", bufs=2) as eh_pool,
            tc.tile_pool(name="ot", bufs=2) as ot_pool,
            tc.tile_pool(name="norm", bufs=2) as norm_pool,
            tc.tile_pool(name="ct", bufs=4) as ct_pool,
            tc.tile_pool(name="ctmp", bufs=2) as ctmp_pool,
        ):
            # ---- static weights ----
            was, wvs = [], []
            for j in range(HL):
                wa = w_pool.tile([128, 128], bf16, tag="w", name=f"wa{j}")
                wvt = w_pool.tile([128, D + 1], bf16, tag="w", name=f"wv{j}")
                nc.sync.dma_start(out=wa[:, :], in_=wa_d[j])
                nc.sync.dma_start(out=wvt[:, :], in_=wv_d[j])
                was.append(wa)
                wvs.append(wvt)
            wots = []
            for p in range(4):
                wt = wot_pool.tile([128, E], bf16, tag="wot", name=f"wot{p}")
                nc.sync.dma_start(out=wt[:, :], in_=wot_d[p * 128:(p + 1) * 128, :])
                wots.append(wt)

            As = [None] * HL
            Bs = [None] * HL
            Vs = [None] * HL
            xts = [None] * HL
            cts = [ct_pool.tile([128, S], bf16, tag="ct", name=f"ct{p}")
                   for p in range(HL // 2)]

            with (
                tc.tile_pool(name="psP", bufs=1, space="PSUM") as psP,
                tc.tile_pool(name="psQ", bufs=1, space="PSUM") as psQ,
                tc.tile_pool(name="psav", bufs=1, space="PSUM") as psav,
            ):
                def load_xt(j):
                    xts[j] = xt_pool.tile([128, S], bf16, tag="xt",
                                          name=f"xt{j}")
                    nc.sync.dma_start(out=xts[j][:, :], in_=xt_d[j])

                def emit_qkv(j, part):
                    """part 0: A blocks 0-2; 1: A block 3 + B swap; 2: V."""
                    if part == 0:
                        As[j] = ab_pool.tile([128, S], bf16, tag="ab",
                                             name=f"A{j}")
                        qa = psQ.tile([128, 1536], f32, tag="psQ",
                                      name=f"qa{j}")
                        for blk in range(3):
                            nc.tensor.matmul(
                                qa[:, blk * 512:(blk + 1) * 512], was[j][:, :],
                                xts[j][:, blk * 512:(blk + 1) * 512])
                        nc.vector.tensor_copy(As[j][:, 0:1536], qa[:, :])
                    elif part == 1:
                        qb = psQ.tile([128, 1536], f32, tag="psQ",
                                      name=f"qb{j}")
                        nc.tensor.matmul(qb[:, 0:512], was[j][:, :],
                                         xts[j][:, 1536:2048])
                        nc.vector.tensor_copy(As[j][:, 1536:2048],
                                              qb[:, 0:512])
                        Bs[j] = ab_pool.tile([128, S], bf16, tag="ab",
                                             name=f"B{j}")
                        nc.sync.dma_start(out=Bs[j][0:64, :],
                                          in_=As[j][64:128, :])
                        nc.sync.dma_start(out=Bs[j][64:128, :],
                                          in_=As[j][0:64, :])
                    else:
                        # V psums: 7 tiles of 65 cols per 512-col bank so no
                        # matmul output crosses a PSUM bank boundary.
                        Vs[j] = v_pool.tile([128, NT * (D + 1)], bf16,
                                            tag="v", name=f"V{j}")
                        qv = psQ.tile([128, 1536], f32, tag="psQ",
                                      name=f"qv{j}")
                        for tt in range(NT):
                            off = (tt // 7) * 512 + (tt % 7) * (D + 1)
                            nc.tensor.matmul(
                                qv[:, off:off + (D + 1)],
                                xts[j][:, tt * 128:(tt + 1) * 128],
                                wvs[j][:, :])
                        for bk in range(3):
                            nb = min(7, NT - bk * 7)
                            nc.vector.tensor_copy(
                                Vs[j][:, bk * 7 * (D + 1):
                                      (bk * 7 + nb) * (D + 1)],
                                qv[:, bk * 512:bk * 512 + nb * (D + 1)])

                # head 0 qkv upfront
                load_xt(0)
                for part in range(3):
                    emit_qkv(0, part)

                for j in range(HL):
                    if j + 1 < HL:
                        load_xt(j + 1)
                    oT = ot_pool.tile([D + 1, S], f32, tag="ot",
                                      name=f"oT{j}")
                    for c in range(NC_CHUNK):
                        sl = slice(c * 512, (c + 1) * 512)
                        av = psav.tile([D + 1, 512], f32, tag="psav",
                                       name=f"av{j}_{c}")
                        # region tiles for this chunk
                        gtiles = []
                        tt0 = 0
                        for gi, (ntl, reg, eng) in enumerate(GROUPS):
                            w = ntl * 512
                            pool = psQ if reg == "Q" else psP
                            gt = pool.tile([128, w], f32,
                                           tag="psQ" if reg == "Q" else "psP",
                                           name=f"g{j}_{c}_{gi}")
                            gtiles.append((gt, ntl, tt0, eng, w))
                            tt0 += ntl

                        # emit: scores for group gi, then exp(gi-1 attnV),...
                        # order: mm(G0), exp(G0), mm(G1), av(G0), exp(G1), ...
                        def pair_mm(ta, tb, ap_a, ap_b):
                            """ta -> ap_a via T0 tile, tb -> ap_b via T8."""
                            nc.tensor.matmul(
                                ap_a, Bs[j][0:64, ta * 128:(ta + 1) * 128],
                                As[j][0:64, sl], tile_position=(0, 0))
                            nc.tensor.matmul(
                                ap_b, As[j][64:128, tb * 128:(tb + 1) * 128],
                                Bs[j][64:128, sl], tile_position=(64, 0))

                        # score mm emission with cross-group pairs:
                        # pairs: (0,1)(2,3) | (4,5)(6,7) | (8,9)(10,11) |
                        #        (12,13) | (14,15); slot of t within groups:
                        def slot_ap(tt):
                            for gt, ntl, t0, eng, w in gtiles:
                                if t0 <= tt < t0 + ntl:
                                    k = tt - t0
                                    return gt[:, k * 512:(k + 1) * 512]
                            raise AssertionError

                        egs = []

                        def emit_exp(gi):
                            gt, ntl, t0, eng, w = gtiles[gi]
                            eg = eg_pool.tile([128, w], bf16, tag="eg",
                                              name=f"eg{j}_{c}_{gi}")
                            if eng == "act":
                                nc.scalar.activation(
                                    eg[:, :], gt[:, :],
                                    mybir.ActivationFunctionType.Exp,
                                    scale=ACT_SCALE)
                            else:
                                # DVE: exp(s/16) in fp32, then one gpsimd
                                # square -> exp(s/8) bf16
                                eh = eh_pool.tile([128, w], f32, tag="eh",
                                                  name=f"eh{j}_{c}_{gi}")
                                nc.vector._custom_dve(
                                    exp4, out=eh[:, :], in0=gt[:, :],
                                    s0=EXP4_C3, s1=EXP4_C2, imm2=EXP4_C1)
                                nc.gpsimd.tensor_tensor(
                                    eg[:, :], eh[:, :], eh[:, :],
                                    op=mybir.AluOpType.mult)
                            egs.append((eg, ntl, t0))

                        def emit_av(gi):
                            eg, ntl, t0 = egs[gi]
                            for k in range(ntl):
                                tt = t0 + k
                                nc.tensor.matmul(
                                    av[:, :],
                                    Vs[j][:, tt * (D + 1):(tt + 1) * (D + 1)],
                                    eg[:, k * 512:(k + 1) * 512],
                                    start=(tt == 0), stop=(tt == NT - 1))

                        # G0 scores
                        pair_mm(0, 1, slot_ap(0), slot_ap(1))
                        pair_mm(2, 3, slot_ap(2), slot_ap(3))
                        emit_exp(0)
                        # G1 scores (pair (6,7) crosses into G2's region)
                        pair_mm(4, 5, slot_ap(4), slot_ap(5))
                        pair_mm(6, 7, slot_ap(6), slot_ap(7))
                        emit_av(0)
                        emit_exp(1)
                        # G2 scores
                        pair_mm(8, 9, slot_ap(8), slot_ap(9))
                        pair_mm(10, 11, slot_ap(10), slot_ap(11))
                        emit_av(1)
                        emit_exp(2)
                        # G3 scores
                        pair_mm(12, 13, slot_ap(12), slot_ap(13))
                        emit_av(2)
                        emit_exp(3)
                        # G4 scores
                        pair_mm(14, 15, slot_ap(14), slot_ap(15))
                        emit_av(3)
                        emit_exp(4)
                        emit_av(4)

                        # evict av chunk into oT
                        nc.vector.tensor_copy(oT[:, sl], av[:, :])

                        # interleave next head's qkv
                        if j + 1 < HL and c < 3:
                            emit_qkv(j + 1, c)

                    # ---- normalization for head j ----
                    cs128 = norm_pool.tile([128, S // 128], f32, tag="cs")
                    rc128 = norm_pool.tile([128, S // 128], f32, tag="rc")
                    bcast = norm_pool.tile([D, S], f32, tag="bc")
                    nc.sync.dma_start(out=cs128[:, :], in_=oT[D:D + 1, :])
                    nc.vector.reciprocal(rc128[:, :], cs128[:, :])
                    nc.sync.dma_start(out=recip_d[j], in_=rc128[:, :])
                    nc.sync.dma_start(
                        out=bcast[:, :],
                        in_=recip_d[j].unsqueeze(0).broadcast_to((D, S)))
                    ct = cts[j // 2]
                    if j % 2 == 0:
                        nc.gpsimd.tensor_tensor(
                            ct[0:D, :], oT[0:D, :], bcast[:, :],
                            op=mybir.AluOpType.mult)
                    else:
                        dst = ctmp_pool.tile([D, S], bf16, tag="ctmp")
                        nc.gpsimd.tensor_tensor(
                            dst[:, :], oT[0:D, :], bcast[:, :],
                            op=mybir.AluOpType.mult)
                        nc.sync.dma_start(out=ct[D:2 * D, :], in_=dst[:, :])

            # ---- output projection ----
            with (
                tc.tile_pool(name="pj_ps", bufs=3, space="PSUM") as pj_ps,
                tc.tile_pool(name="po", bufs=3) as po_pool,
            ):
                for sc in range(S // 128):
                    pso = pj_ps.tile([128, E], f32, tag="pj")
                    for p in range(4):
                        for half in range(2):
                            hsl = slice(half * 512, (half + 1) * 512)
                            nc.tensor.matmul(
                                pso[:, hsl],
                                cts[p][:, sc * 128:(sc + 1) * 128],
                                wots[p][:, hsl],
                                start=(p == 0), stop=(p == 3))
                    osb = po_pool.tile([128, E], f32, tag="po")
                    nc.vector.tensor_copy(osb[:, :], pso[:, :])
                    nc.sync.dma_start(out=out_d[sc * 128:(sc + 1) * 128, :],
                                      in_=osb[:, :])

    nc.compile()
    return nc


def prep_inputs(token_encodings, Wq, Wk, Wv, bq, bk, bv, Wo, bo):
    """Build per-core input maps. Core c = b*2+g."""
    x = np.asarray(token_encodings, dtype=np.float32)
    wq = np.asarray(Wq, np.float32)
    wk = np.asarray(Wk, np.float32)
    wv = np.asarray(Wv, np.float32)
    bq_ = np.asarray(bq, np.float32)
    bk_ = np.asarray(bk, np.float32)
    bv_ = np.asarray(bv, np.float32)
    wo = np.asarray(Wo, np.float32)
    maps = []
    for c in range(N_CORES):
        b, g = divmod(c, 2)
        xt_full = np.ascontiguousarray(x[b].T)  # (E, S)
        xt = np.zeros((HL, 128, S), dtype=BF16)
        wa = np.zeros((HL, 128, 128), dtype=BF16)
        wv_a = np.zeros((HL, 128, D + 1), dtype=BF16)
        for j in range(HL):
            h = g * HL + j
            xt[j, :D] = xt_full[h * D:(h + 1) * D].astype(BF16)
            xt[j, D] = np.float32(1.0)
            # A-stationary: cols 0:64 -> Q' (scaled), cols 64:128 -> K
            wa[j, :D, :D] = (wq[h] * SIG).astype(BF16)
            wa[j, D, :D] = (bq_[h] * SIG).astype(BF16)
            wa[j, :D, D:2 * D] = wk[h].astype(BF16)
            wa[j, D, D:2 * D] = bk_[h].astype(BF16)
            wv_a[j, :D, :D] = wv[h].astype(BF16)
            wv_a[j, D, :D] = bv_[h].astype(BF16)
            wv_a[j, D, D] = np.float32(1.0)
        wot = np.ascontiguousarray(
            wo[:, g * 512:(g + 1) * 512].T).astype(BF16)
        maps.append({"xt": xt, "wa": wa, "wv": wv_a, "wot": wot})
    return maps


def kernel(**inputs):
    from concourse.bass_utils import run_bass_kernel_spmd

    if "nc" not in _CACHE:
        _CACHE["nc"] = build_nc()
    nc = _CACHE["nc"]
    in_maps = prep_inputs(**inputs)
    res = run_bass_kernel_spmd(nc, in_maps, list(range(N_CORES)))
    bo_f = np.asarray(inputs["bo"], np.float32)
    out = np.empty((B, S, E), dtype=np.float32)
    for b in range(B):
        out[b] = res.results[2 * b]["out"] + res.results[2 * b + 1]["out"] + bo_f
    return out


# revision 12
# speedup vs baseline: 1.2339x; 1.2339x over previous
"""Trainium2 Bass kernel: multi-head attention (B=4, S=2048, E=1024, H=16, D=64).

Sharding: 8 cores = 4 batches x 2 head-groups (8 heads each). Each core
computes attention for its (batch, 8-head group) and a partial output
projection over its 512 channels; the host sums the two partials per batch
and adds the output bias.

Per-core dataflow (bf16 matmuls, fp32 PSUM):
  A[j] = wA[j].T @ XT_aug[j]    -> [Q'^T (rows 0:64) ; K^T (rows 64:128)]
         where Q' = (x Wq + bq) * log2(e)/32  (scale folded on host)
  B[j] = partition-swapped copy of A[j] (SBUF->SBUF DMA) = [K^T ; Q'^T]
  scores^T for two t-tiles at once via PE row tiling (K=64):
         T0 tile (0,0):  B[0:64,tcols].T @ A[0:64,s]   (t even slot)
         T8 tile (64,0): A[64:128,tcols].T @ B[64:128,s] (t odd slot)
  psum w = scores * log2(e)/32; exp(s/8) = 2^(4w):
         ScalarE groups: Exp activation, scale = 4*ln2
         DVE groups:     custom 8-stage op (cubic 2^w then 2 squarings);
                         global scale a0^4 cancels in softmax
  attnV: av[65,512] += V_aug[tt].T @ exp_tile  (ones col -> denominator)
  normalize: recip via DMA-bounce broadcast, multiply on GpSimd
  out partial = concatT.T @ WoT -> [s,1024] fp32 -> DRAM
"""

import sys

sys.path.insert(0, "/opt/trn_rl_repo")

import numpy as np
import ml_dtypes

BF16 = ml_dtypes.bfloat16

B, S, E, H = 4, 2048, 1024, 16
D = E // H          # 64
HL = 8              # heads per core
N_CORES = 8
NT = S // 128       # 16 t-tiles
NC_CHUNK = 4        # s-chunks of 512
SIG = float(np.log2(np.e) / 64.0)   # score scale folded into Wq

# custom DVE op: P(w)=((c3 w + c2) w + c1) w + 1 ; out = (P^2)^2 ~ 2^(4w)/a0^4
# kernel folds SIG=log2e/64 into Wq, so DVE out = exp(s/16)/a0^4; a final
# gpsimd square yields exp(s/8)/a0^8 (the a0^8 cancels in softmax).
EXP4_NAME = "ANT_EXP4_SOFTMAX"
EXP4_C3 = 0.0558784277
EXP4_C2 = 0.242307174
EXP4_C1 = 0.693163145
EXP4_A0 = 0.99994823
EXP4_SCALE = EXP4_A0 ** 4   # DVE output is 2^(4w)/EXP4_SCALE

# group structure per s-chunk: (ntiles, region, engine)
# regions: 'P' (banks 0-3, [128,2048]), 'Q' (banks 4-6, [128,1536]),
#          'P2' ([128,1024] from P pool)
# DVE-path group first: its longer exp latency (DVE poly + gpsimd square)
# overlaps the ScalarE processing of the later groups.
GROUPS = [
    (4, "P", "dve"),    # t0-3
    (3, "Q", "act"),    # t4-6
    (4, "P", "act"),    # t7-10
    (3, "Q", "act"),    # t11-13
    (2, "P2", "act"),   # t14,15
]
# attnV emission order over groups (first emitted = accumulation start,
# last = stop): DVE group in the middle, fast ACT group last.
AV_ORDER = [1, 2, 0, 3, 4]

_CACHE = {}


def register_exp4():
    import concourse.dve_ops as dmod
    from concourse.dve_spec import Spec, Src0, C0, C1, C2, One, sq, lower
    from concourse.dve_uop import DveOpSpec

    for op in dmod.OPS:
        if op.name == EXP4_NAME:
            return op

    body = sq(sq(((C0 * Src0 + C1) * Src0 + C2) * Src0 + One))

    def _ref(in0, in1, s0, s1, imm2):
        x = in0.astype(np.float32)
        p = (((s0 * x + s1) * x + imm2) * x + np.float32(1.0)).astype(np.float32)
        p2 = (p * p).astype(np.float32)
        return (p2 * p2).astype(np.float32)

    spec = Spec(body=body, reference=_ref)
    row = dmod._CUSTOM_DVE_ROW_BASE + len(dmod.OPS)
    assert row < 0x20
    shas = {}
    for ver in ("v3", "v4"):
        try:
            uops = lower(spec, ver=ver)
            shas[ver] = DveOpSpec(
                name=EXP4_NAME, opcode=row, uops=uops, rd1_en=False
            ).sha(ver)
        except Exception:
            pass
    op = dmod.DveOp(EXP4_NAME, spec, subdim=False, uops_sha=shas)
    dmod.OPS.append(op)
    dmod.CUSTOM_DVE_SPECS[EXP4_NAME] = spec
    dmod._SUB_OPCODE_FOR_NAME[EXP4_NAME] = row
    return op


def build_nc():
    import concourse.mybir as mybir
    import concourse.tile as tile
    from concourse import bacc

    f32 = mybir.dt.float32
    bf16 = mybir.dt.bfloat16
    exp4 = register_exp4()
    ACT_SCALE = float(8.0 * np.log(2.0))

    nc = bacc.Bacc(None)

    xt_d = nc.dram_tensor("xt", [HL, 128, S], bf16, kind="ExternalInput")
    wa_d = nc.dram_tensor("wa", [HL, 128, 128], bf16, kind="ExternalInput")
    wv_d = nc.dram_tensor("wv", [HL, 128, D + 1], bf16, kind="ExternalInput")
    wot_d = nc.dram_tensor("wot", [HL * D, E], bf16, kind="ExternalInput")
    out_d = nc.dram_tensor("out", [S, E], f32, kind="ExternalOutput")
    recip_d = nc.dram_tensor("recip_dram", [HL, S], f32)

    with tile.TileContext(nc) as tc:
        with (
            tc.tile_pool(name="xt", bufs=3) as xt_pool,
            tc.tile_pool(name="w", bufs=2 * HL) as w_pool,
            tc.tile_pool(name="ab", bufs=6) as ab_pool,
            tc.tile_pool(name="v", bufs=3) as v_pool,
            tc.tile_pool(name="wot", bufs=4) as wot_pool,
            tc.tile_pool(name="eg", bufs=6) as eg_pool,
            tc.tile_pool(name="eh", bufs=2) as eh_pool,
            tc.tile_pool(name="ot", bufs=2) as ot_pool,
            tc.tile_pool(name="norm", bufs=2) as norm_pool,
            tc.tile_pool(name="ct", bufs=4) as ct_pool,
            tc.tile_pool(name="ctmp", bufs=2) as ctmp_pool,
        ):
            # ---- static weights ----
            was, wvs = [], []
            for j in range(HL):
                wa = w_pool.tile([128, 128], bf16, tag="w", name=f"wa{j}")
                wvt = w_pool.tile([128, D + 1], bf16, tag="w", name=f"wv{j}")
                nc.sync.dma_start(out=wa[:, :], in_=wa_d[j])
                nc.sync.dma_start(out=wvt[:, :], in_=wv_d[j])
                was.append(wa)
                wvs.append(wvt)
            wots = []
            for p in range(4):
                wt = wot_pool.tile([128, E], bf16, tag="wot", name=f"wot{p}")
                nc.sync.dma_start(out=wt[:, :], in_=wot_d[p * 128:(p + 1) * 128, :])
                wots.append(wt)

            As = [None] * HL
            Bs = [None] * HL
            Vs = [None] * HL
            xts = [None] * HL
            cts = [ct_pool.tile([128, S], bf16, tag="ct", name=f"ct{p}")
                   for p in range(HL // 2)]

            with (
                tc.tile_pool(name="psP", bufs=1, space="PSUM") as psP,
                tc.tile_pool(name="psQ", bufs=1, space="PSUM") as psQ,
                tc.tile_pool(name="psav", bufs=1, space="PSUM") as psav,
            ):
                def load_xt(j):
                    xts[j] = xt_pool.tile([128, S], bf16, tag="xt",
                                          name=f"xt{j}")
                    nc.sync.dma_start(out=xts[j][:, :], in_=xt_d[j])

                def emit_qkv(j, part):
                    """part 0: A blocks 0-2; 1: A block 3 + B swap; 2: V."""
                    if part == 0:
                        As[j] = ab_pool.tile([128, S], bf16, tag="ab",
                                             name=f"A{j}")
                        qa = psQ.tile([128, 1536], f32, tag="psQ",
                                      name=f"qa{j}")
                        for blk in range(3):
                            nc.tensor.matmul(
                                qa[:, blk * 512:(blk + 1) * 512], was[j][:, :],
                                xts[j][:, blk * 512:(blk + 1) * 512])
                        nc.vector.tensor_copy(As[j][:, 0:1536], qa[:, :])
                    elif part == 1:
                        qb = psQ.tile([128, 1536], f32, tag="psQ",
                                      name=f"qb{j}")
                        nc.tensor.matmul(qb[:, 0:512], was[j][:, :],
                                         xts[j][:, 1536:2048])
                        nc.vector.tensor_copy(As[j][:, 1536:2048],
                                              qb[:, 0:512])
                        Bs[j] = ab_pool.tile([128, S], bf16, tag="ab",
                                             name=f"B{j}")
                        nc.sync.dma_start(out=Bs[j][0:64, :],
                                          in_=As[j][64:128, :])
                        nc.sync.dma_start(out=Bs[j][64:128, :],
                                          in_=As[j][0:64, :])
                    else:
                        # V psums: 7 tiles of 65 cols per 512-col bank so no
                        # matmul output crosses a PSUM bank boundary.
                        Vs[j] = v_pool.tile([128, NT * (D + 1)], bf16,
                                            tag="v", name=f"V{j}")
                        qv = psQ.tile([128, 1536], f32, tag="psQ",
                                      name=f"qv{j}")
                        for tt in range(NT):
                            off = (tt // 7) * 512 + (tt % 7) * (D + 1)
                            nc.tensor.matmul(
                                qv[:, off:off + (D + 1)],
                                xts[j][:, tt * 128:(tt + 1) * 128],
                                wvs[j][:, :])
                        for bk in range(3):
                            nb = min(7, NT - bk * 7)
                            nc.vector.tensor_copy(
                                Vs[j][:, bk * 7 * (D + 1):
                                      (bk * 7 + nb) * (D + 1)],
                                qv[:, bk * 512:bk * 512 + nb * (D + 1)])

                # head 0 qkv upfront
                load_xt(0)
                for part in range(3):
                    emit_qkv(0, part)

                for j in range(HL):
                    if j + 1 < HL:
                        load_xt(j + 1)
                    oT = ot_pool.tile([D + 1, S], f32, tag="ot",
                                      name=f"oT{j}")
                    for c in range(NC_CHUNK):
                        sl = slice(c * 512, (c + 1) * 512)
                        av = psav.tile([D + 1, 512], f32, tag="psav",
                                       name=f"av{j}_{c}")
                        # region tiles for this chunk
                        gtiles = []
                        tt0 = 0
                        for gi, (ntl, reg, eng) in enumerate(GROUPS):
                            w = ntl * 512
                            pool = psQ if reg == "Q" else psP
                            gt = pool.tile([128, w], f32,
                                           tag="psQ" if reg == "Q" else "psP",
                                           name=f"g{j}_{c}_{gi}")
                            gtiles.append((gt, ntl, tt0, eng, w))
                            tt0 += ntl

                        # emit: scores for group gi, then exp(gi-1 attnV),...
                        # order: mm(G0), exp(G0), mm(G1), av(G0), exp(G1), ...
                        def pair_mm(ta, tb, ap_a, ap_b):
                            """ta -> ap_a via T0 tile, tb -> ap_b via T8."""
                            nc.tensor.matmul(
                                ap_a, Bs[j][0:64, ta * 128:(ta + 1) * 128],
                                As[j][0:64, sl], tile_position=(0, 0))
                            nc.tensor.matmul(
                                ap_b, As[j][64:128, tb * 128:(tb + 1) * 128],
                                Bs[j][64:128, sl], tile_position=(64, 0))

                        # score mm emission with cross-group pairs:
                        # pairs: (0,1)(2,3) | (4,5)(6,7) | (8,9)(10,11) |
                        #        (12,13) | (14,15); slot of t within groups:
                        def slot_ap(tt):
                            for gt, ntl, t0, eng, w in gtiles:
                                if t0 <= tt < t0 + ntl:
                                    k = tt - t0
                                    return gt[:, k * 512:(k + 1) * 512]
                            raise AssertionError

                        egs = {}
                        av_n = [0]

                        def emit_exp(gi):
                            gt, ntl, t0, eng, w = gtiles[gi]
                            eg = eg_pool.tile([128, w], bf16, tag="eg",
                                              name=f"eg{j}_{c}_{gi}")
                            if eng == "act":
                                nc.scalar.activation(
                                    eg[:, :], gt[:, :],
                                    mybir.ActivationFunctionType.Exp,
                                    scale=ACT_SCALE)
                            else:
                                # DVE: exp(s/16) in fp32, then one gpsimd
                                # square -> exp(s/8) bf16
                                eh = eh_pool.tile([128, w], f32, tag="eh",
                                                  name=f"eh{j}_{c}_{gi}")
                                nc.vector._custom_dve(
                                    exp4, out=eh[:, :], in0=gt[:, :],
                                    s0=EXP4_C3, s1=EXP4_C2, imm2=EXP4_C1)
                                nc.gpsimd.tensor_tensor(
                                    eg[:, :], eh[:, :], eh[:, :],
                                    op=mybir.AluOpType.mult)
                            egs[gi] = (eg, ntl, t0)

                        def emit_av(gi):
                            eg, ntl, t0 = egs[gi]
                            for k in range(ntl):
                                tt = t0 + k
                                nc.tensor.matmul(
                                    av[:, :],
                                    Vs[j][:, tt * (D + 1):(tt + 1) * (D + 1)],
                                    eg[:, k * 512:(k + 1) * 512],
                                    start=(av_n[0] == 0),
                                    stop=(av_n[0] == NT - 1))
                                av_n[0] += 1

                        # G0 scores (DVE-path exp starts the chunk)
                        pair_mm(0, 1, slot_ap(0), slot_ap(1))
                        pair_mm(2, 3, slot_ap(2), slot_ap(3))
                        emit_exp(0)
                        # G1 scores (pair (6,7) crosses into G2's region)
                        pair_mm(4, 5, slot_ap(4), slot_ap(5))
                        pair_mm(6, 7, slot_ap(6), slot_ap(7))
                        emit_exp(1)
                        emit_av(1)
                        # G2 scores
                        pair_mm(8, 9, slot_ap(8), slot_ap(9))
                        pair_mm(10, 11, slot_ap(10), slot_ap(11))
                        emit_exp(2)
                        emit_av(2)
                        # G3 scores
                        pair_mm(12, 13, slot_ap(12), slot_ap(13))
                        emit_exp(3)
                        emit_av(0)
                        # G4 scores
                        pair_mm(14, 15, slot_ap(14), slot_ap(15))
                        emit_exp(4)
                        emit_av(3)
                        emit_av(4)

                        # evict av chunk into oT
                        nc.vector.tensor_copy(oT[:, sl], av[:, :])

                        # interleave next head's qkv
                        if j + 1 < HL and c < 3:
                            emit_qkv(j + 1, c)

                    # ---- normalization for head j ----
                    cs128 = norm_pool.tile([128, S // 128], f32, tag="cs")
                    rc128 = norm_pool.tile([128, S // 128], f32, tag="rc")
                    bcast = norm_pool.tile([D, S], f32, tag="bc")
                    nc.sync.dma_start(out=cs128[:, :], in_=oT[D:D + 1, :])
                    nc.vector.reciprocal(rc128[:, :], cs128[:, :])
                    nc.sync.dma_start(out=recip_d[j], in_=rc128[:, :])
                    nc.sync.dma_start(
                        out=bcast[:, :],
                        in_=recip_d[j].unsqueeze(0).broadcast_to((D, S)))
                    ct = cts[j // 2]
                    if j % 2 == 0:
                        nc.gpsimd.tensor_tensor(
                            ct[0:D, :], oT[0:D, :], bcast[:, :],
                            op=mybir.AluOpType.mult)
                    else:
                        dst = ctmp_pool.tile([D, S], bf16, tag="ctmp")
                        nc.gpsimd.tensor_tensor(
                            dst[:, :], oT[0:D, :], bcast[:, :],
                            op=mybir.AluOpType.mult)
                        nc.sync.dma_start(out=ct[D:2 * D, :], in_=dst[:, :])

            # ---- output projection ----
            with (
                tc.tile_pool(name="pj_ps", bufs=3, space="PSUM") as pj_ps,
                tc.tile_pool(name="po", bufs=3) as po_pool,
            ):
                for sc in range(S // 128):
                    pso = pj_ps.tile([128, E], f32, tag="pj")
                    for p in range(4):
                        for half in range(2):
                            hsl = slice(half * 512, (half + 1) * 512)
                            nc.tensor.matmul(
                                pso[:, hsl],
                                cts[p][:, sc * 128:(sc + 1) * 128],
                                wots[p][:, hsl],
                                start=(p == 0), stop=(p == 3))
                    osb = po_pool.tile([128, E], f32, tag="po")
                    nc.vector.tensor_copy(osb[:, :], pso[:, :])
                    nc.sync.dma_start(out=out_d[sc * 128:(sc + 1) * 128, :],
                                      in_=osb[:, :])

    nc.compile()
    return nc


def prep_inputs(token_encodings, Wq, Wk, Wv, bq, bk, bv, Wo, bo):
    """Build per-core input maps. Core c = b*2+g."""
    x = np.asarray(token_encodings, dtype=np.float32)
    wq = np.asarray(Wq, np.float32)
    wk = np.asarray(Wk, np.float32)
    wv = np.asarray(Wv, np.float32)
    bq_ = np.asarray(bq, np.float32)
    bk_ = np.asarray(bk, np.float32)
    bv_ = np.asarray(bv, np.float32)
    wo = np.asarray(Wo, np.float32)
    maps = []
    for c in range(N_CORES):
        b, g = divmod(c, 2)
        xt_full = np.ascontiguousarray(x[b].T)  # (E, S)
        xt = np.zeros((HL, 128, S), dtype=BF16)
        wa = np.zeros((HL, 128, 128), dtype=BF16)
        wv_a = np.zeros((HL, 128, D + 1), dtype=BF16)
        for j in range(HL):
            h = g * HL + j
            xt[j, :D] = xt_full[h * D:(h + 1) * D].astype(BF16)
            xt[j, D] = np.float32(1.0)
            # A-stationary: cols 0:64 -> Q' (scaled), cols 64:128 -> K
            wa[j, :D, :D] = (wq[h] * SIG).astype(BF16)
            wa[j, D, :D] = (bq_[h] * SIG).astype(BF16)
            wa[j, :D, D:2 * D] = wk[h].astype(BF16)
            wa[j, D, D:2 * D] = bk_[h].astype(BF16)
            wv_a[j, :D, :D] = wv[h].astype(BF16)
            wv_a[j, D, :D] = bv_[h].astype(BF16)
            wv_a[j, D, D] = np.float32(1.0)
        wot = np.ascontiguousarray(
            wo[:, g * 512:(g + 1) * 512].T).astype(BF16)
        maps.append({"xt": xt, "wa": wa, "wv": wv_a, "wot": wot})
    return maps


def kernel(**inputs):
    from concourse.bass_utils import run_bass_kernel_spmd

    if "nc" not in _CACHE:
        _CACHE["nc"] = build_nc()
    nc = _CACHE["nc"]
    in_maps = prep_inputs(**inputs)
    res = run_bass_kernel_spmd(nc, in_maps, list(range(N_CORES)))
    bo_f = np.asarray(inputs["bo"], np.float32)
    out = np.empty((B, S, E), dtype=np.float32)
    for b in range(B):
        out[b] = res.results[2 * b]["out"] + res.results[2 * b + 1]["out"] + bo_f
    return out
